# revision 22
# baseline (speedup 1.0000x reference)
"""Trainium2 Bass kernel for nn_DepPairingLayer (bidirectional chain-TreeLSTM over
shortest-path node chains + span mean-pooling + pair MLP), SPMD across 8 NeuronCores.

Sharding: data-parallel over the pair dimension P=8192 (1024 pairs/core = 4
batches x 256 pairs); all weights replicated.

Layout is feature-major: activations live as [features(partitions), pairs(free)].
Both the input projection x @ W and the recurrence h @ U run in fp8-e4m3 with
MatmulPerfMode.DoubleRow; the combined 1216-row contraction (832 node features +
384 hidden) is packed into 5 DoubleRow pairs per gate m-tile:
  pairs 0-2: x rows 0:768 (DMA-fed),
  pair 3:    block0 = x rows 768:832 on partitions 0:64 (stationary rows 64:128
             are zero so the moving garbage there is nullified), block1 = h rows
             256:384 (written in place by the vector engine each step),
  pair 4:    block0/1 = h rows 0:128 / 128:256.
All weights are scaled by 64 host-side (power of two) so their 0.02-sigma
entries land in e4m3's normal range; the 64x is divided back out in the gate
activation's scale. Cell state c and all captures (root/start/end) stay bf16.

Pairs are permuted host-side so that, within each batch, pairs are sorted by
root_idx; physical column order is (half, batch, rank) so chunk 0 holds every
batch's low-root half. The up-direction LSTM then runs only s_up[ch] steps per
chunk (s_up computed from the actual root_idx at kernel() time; the program is
compiled per schedule). Span pooling stays batch-major internally and its
results are written into the permuted column slots. Output is inverse-permuted
host-side.
"""

from contextlib import ExitStack

import numpy as np
import ml_dtypes

import concourse.bass as bass
import concourse.mybir as mybir
import concourse.tile as tile
from concourse import bacc
from concourse.bass_utils import run_bass_kernel_spmd
from concourse.masks import make_identity

bf16 = ml_dtypes.bfloat16
fp8 = ml_dtypes.float8_e4m3
FP32 = mybir.dt.float32
BF16 = mybir.dt.bfloat16
FP8 = mybir.dt.float8e4
ALU = mybir.AluOpType
ACTF = mybir.ActivationFunctionType
DR = mybir.MatmulPerfMode.DoubleRow

# problem dims (hardcoded per contract)
NCORES = 8
B, PB, L, D, H, DT, T = 32, 256, 16, 832, 384, 768, 512
P = B * PB                      # 8192 pairs
PS = P // NCORES                # 1024 pairs per core
NB = B // NCORES                # 4 batches per core
C = 512                         # pair-chunk (matmul moving free dim)
NCH = PS // C                   # 2 chunks per core
H4 = 4 * H                      # 1536 = i|o|u|f
WSC = 64.0                      # fp8 weight pre-scale (power of two)
# x(832) + h(384) k-rows packed as 5 DoubleRow pairs (see module docstring)
KPA = 3                         # full 256-row DR pairs of node features
KP5 = 5                         # total DR pairs incl mixed x-tail/h pairs
M12 = H4 // 128                 # 12 m-tiles of gate features
KH = H // 128                   # 3 k-tiles of hidden
DEC_IN, DEC_H, DEC_OUT = 3 * H + 2 * DT, 512, 7
K21 = DEC_IN // 128             # 21 feature k-tiles for W1
M4 = DEC_H // 128               # 4 m-tiles for W1 output
MT = DT // 128                  # 6 span-feature m-tiles
JT = PB // 128                  # 2 pair-tiles per batch (for masks)


def _build_program(s_up=(L, L), debug: bool = False, loop_n: int = 0,
                   probe: str = "") -> bass.Bass:
    """s_up[ch]: number of up-direction steps chunk ch needs (max root + 1 over
    that chunk's columns, same for every core). loop_n > 0 wraps the body in a
    For_i timing loop (identical work each iteration)."""
    nc = bacc.Bacc("TRN2", target_bir_lowering=False, debug=False,
                   num_devices=NCORES)
    dp = nc.declare_dram_parameter

    node_a = dp("node_a", [L, KPA, 128, 2, PS], FP8, isOutput=False)
    node_b = dp("node_b", [L, 128, PS], FP8, isOutput=False)
    tok = dp("tok", [NB, T, DT], BF16, isOutput=False)
    root = dp("root", [1, PS], FP32, isOutput=False)
    sp_all = dp("sp_all", [2, NB, JT, 128, 4], FP32, isOutput=False)
    Wua = dp("Wua", [KP5, 128, 2, H4], FP8, isOutput=False)
    Wda = dp("Wda", [KP5, 128, 2, H4], FP8, isOutput=False)
    W1 = dp("W1", [DEC_IN, DEC_H], BF16, isOutput=False)
    W2 = dp("W2", [DEC_H, DEC_OUT], BF16, isOutput=False)
    bu = dp("bu", [M12, 128, 1], FP32, isOutput=False)
    bd = dp("bd", [M12, 128, 1], FP32, isOutput=False)
    b1 = dp("b1", [M4, 128, 1], FP32, isOutput=False)
    b2 = dp("b2", [DEC_OUT, 1], FP32, isOutput=False)
    ones = dp("ones", [1, 128], BF16, isOutput=False)
    iota_d = dp("iota_d", [128, T], FP32, isOutput=False)
    out_d = dp("out", [DEC_OUT, PS], FP32, isOutput=True)

    def loadc(pool, name, src_ap, shape, dtype, bufs=1):
        t = pool.tile(shape, dtype, name=name, tag=name, bufs=bufs)
        nc.sync.dma_start(t[:], src_ap)
        return t

    with tile.TileContext(nc) as tc, ExitStack() as ctx:
        if loop_n:
            ctx.enter_context(tc.For_i(0, loop_n, 1))
        # whole-program pools
        cpool = ctx.enter_context(tc.tile_pool(name="const", bufs=1))
        spanp = ctx.enter_context(tc.tile_pool(name="spanp", bufs=2))
        capp = ctx.enter_context(tc.tile_pool(name="capp", bufs=2))
        pmm = ctx.enter_context(tc.tile_pool(name="pmm", bufs=5, space="PSUM"))
        pzp = ctx.enter_context(tc.tile_pool(name="pzp", bufs=2, space="PSUM"))

        bu_t = [loadc(cpool, f"bu{m}", bu[m], [128, 1], FP32) for m in range(M12)]
        bd_t = [loadc(cpool, f"bd{m}", bd[m], [128, 1], FP32) for m in range(M12)]
        b1_t = [loadc(cpool, f"b1{m}", b1[m], [128, 1], FP32) for m in range(M4)]
        b2_t = loadc(cpool, "b2t", b2[:, :], [DEC_OUT, 1], FP32)
        ones_t = loadc(cpool, "onest", ones[:, :], [1, 128], BF16)
        root_t = loadc(cpool, "roott", root[:, :], [1, PS], FP32)
        iota_t = loadc(cpool, "iota", iota_d[:, :], [128, T], FP32)
        ident = cpool.tile([128, 128], BF16, name="ident", tag="ident")
        make_identity(nc, ident[:])

        b_t = {"u": bu_t, "d": bd_t}

        # spanT[sp][m]: [128, PS] bf16 feature-major span means (whole program)
        spanT = [[spanp.tile([128, PS], BF16, name=f"span{sp}_{m}",
                             tag=f"span{sp}_{m}") for m in range(MT)]
                 for sp in range(2)]
        # per-chunk LSTM summary tiles (whole program; consumed by the MLP
        # phase). fp8: h state is already e4m3-quantized, so this is lossless.
        root_acc = [[capp.tile([128, C], FP8, name=f"racc{ch}_{k}",
                               tag=f"racc{ch}_{k}") for k in range(KH)]
                    for ch in range(NCH)]
        start_t = [[None] * KH for _ in range(NCH)]
        end_t = [[None] * KH for _ in range(NCH)]

        # ---- span mean pooling for one batch (interleaved into the LSTM
        # step loop to fill DVE/PE slack; spanT is only read by the MLP) ----
        tokp = ctx.enter_context(tc.tile_pool(name="tokp", bufs=2))
        mwork = ctx.enter_context(tc.tile_pool(name="mwork", bufs=2))
        ptp = ctx.enter_context(tc.tile_pool(name="ptp", bufs=1, space="PSUM"))

        def emit_span(b):
            tk = []
            for tb in range(T // 128):
                t = tokp.tile([128, DT], BF16, name=f"tok{tb}", tag=f"tok{tb}")
                nc.sync.dma_start(t[:], tok[b, tb * 128:(tb + 1) * 128, :])
                tk.append(t)
            for sp in range(2):
                maskT = [mwork.tile([128, PB], BF16, name=f"mT{tb}",
                                    tag=f"mT{tb}") for tb in range(T // 128)]
                for jt in range(JT):
                    sc3 = mwork.tile([128, 4], FP32, name="sc3", tag="sc3",
                                     bufs=4)
                    nc.sync.dma_start(sc3[:], sp_all[sp, b, jt])
                    cmp1 = mwork.tile([128, T], BF16, name="cmp1", tag="cmp1")
                    cmp2 = mwork.tile([128, T], BF16, name="cmp2", tag="cmp2")
                    nc.vector.tensor_scalar(cmp1[:], iota_t[:], sc3[:, 0:1],
                                            None, ALU.is_ge)
                    nc.vector.tensor_scalar(cmp2[:], iota_t[:], sc3[:, 1:2],
                                            None, ALU.is_lt)
                    m16 = mwork.tile([128, T], BF16, name="m16", tag="m16")
                    nc.vector.scalar_tensor_tensor(m16[:], cmp1[:],
                                                   sc3[:, 2:3], cmp2[:],
                                                   op0=ALU.mult, op1=ALU.mult)
                    for tb in range(T // 128):
                        tp = ptp.tile([128, 128], BF16, name="tp", tag="tp")
                        nc.tensor.transpose(
                            tp[:], m16[:, tb * 128:(tb + 1) * 128], ident[:])
                        nc.vector.tensor_copy(
                            maskT[tb][:, jt * 128:(jt + 1) * 128], tp[:])
                for m in range(MT):
                    zp = pzp.tile([128, PB], FP32, name="zp", tag="zp")
                    for tb in range(T // 128):
                        nc.tensor.matmul(zp[:],
                                         tk[tb][:, m * 128:(m + 1) * 128],
                                         maskT[tb][:], start=(tb == 0),
                                         stop=(tb == T // 128 - 1))
                    # physical columns: lows at 128*b, highs at 512+128*b
                    nc.vector.tensor_copy(
                        spanT[sp][m][:, 128 * b:128 * (b + 1)], zp[:, 0:128])
                    nc.vector.tensor_copy(
                        spanT[sp][m][:, C + 128 * b:C + 128 * (b + 1)],
                        zp[:, 128:256])

        # ---- phase 2: bidirectional chain-LSTM per pair-chunk ----------
        with tc.tile_pool(name="lstmw", bufs=1) as lstmw, \
             tc.tile_pool(name="nodep", bufs=4) as nodep, \
             tc.tile_pool(name="statep", bufs=2) as statep, \
             tc.tile_pool(name="state8", bufs=2) as state8, \
             tc.tile_pool(name="gatep", bufs=24) as gatep, \
             tc.tile_pool(name="eqp", bufs=4) as eqp:
            samew = probe == "samew"
            wa_t = {dd: [loadc(lstmw, f"wa_{dd}{p}", src[p], [128, 2, H4], FP8)
                         for p in range(KP5)]
                    for dd, src in (("u", Wua), ("d", Wda))}

            # fp8 moving state: mx holds [x-tail | h rows 256:384], hp holds
            # h rows 0:256. Allocated one step ahead; h(s) is written into the
            # tiles that step s+1's matmuls read.
            mx = {}
            hp = {}
            cst = {}
            for ch in range(NCH):
                for d in ("u", "d"):
                    mx[d, ch] = state8.tile([128, 2, C], FP8, name=f"mx_{d}_{ch}",
                                            tag=f"mx_{d}_{ch}")
                    hp[d, ch] = state8.tile([128, 2, C], FP8, name=f"hp_{d}_{ch}",
                                            tag=f"hp_{d}_{ch}")
                    nc.vector.memset(hp[d, ch][:], 0.0)
                    nc.vector.memset(mx[d, ch][:, 1, :], 0.0)
                    cst[d, ch] = [statep.tile([128, C], BF16, name=f"c_{d}{k}_{ch}",
                                              tag=f"c_{d}{k}_{ch}")
                                  for k in range(KH)]
                for k in range(KH):
                    nc.vector.memset(root_acc[ch][k][:], 0.0)

            def h_slices(mx_t, hp_t):
                # h k-tile k lives at: k=0 -> hp[:,0], k=1 -> hp[:,1],
                # k=2 -> mx[:,1]
                return [hp_t[:, 0, :], hp_t[:, 1, :], mx_t[:, 1, :]]

            for s in range(L if probe != "nolstm" else 0):
                # one batch's span pooling every other step, riding in the
                # engines' slack while the LSTM runs
                if probe != "nospan" and s % 2 == 1 and s // 2 < NB:
                    emit_span(s // 2)
                for d in ("u", "d"):
                    t_src = s if d == "u" else L - 1 - s
                    # chunks still active at this step for this direction
                    acts = [ch for ch in range(NCH)
                            if d == "d" or s < s_up[ch]]
                    if not acts:
                        continue
                    nda = {}
                    for ch in acts:
                        c0 = ch * C
                        nda[ch] = []
                        for p in range(KPA):
                            t = nodep.tile([128, 2, C], FP8, name=f"nda{p}",
                                           tag=f"nda{p}")
                            nc.sync.dma_start(t[:],
                                              node_a[t_src, p, :, :, c0:c0 + C])
                            nda[ch].append(t)
                        # x-tail rides in the mixed tile's block 0 (rows 64:128
                        # are zero-padded host-side, matching zero stationary)
                        nc.sync.dma_start(mx[d, ch][:, 0, :],
                                          node_b[t_src, :, c0:c0 + C])
                    gates = {ch: [] for ch in acts}
                    for m in range(M12):
                        ms = slice(0, 128) if samew else slice(m * 128,
                                                               (m + 1) * 128)
                        pm = {ch: pmm.tile([128, C], FP32, name="pm", tag="mm")
                              for ch in acts}
                        for p in range(KP5):
                            for ch in acts:
                                mov = (nda[ch][p][:] if p < KPA else
                                       mx[d, ch][:] if p == KPA else
                                       hp[d, ch][:])
                                wsl = wa_t[d][0 if samew else p][:, :, ms]
                                nc.tensor.matmul(pm[ch][:], wsl, mov,
                                                 start=(p == 0),
                                                 stop=(p == KP5 - 1),
                                                 perf_mode=DR)
                        for ch in acts:
                            g = gatep.tile([128, C], BF16, name="g", tag="g")
                            func = (ACTF.Tanh if 2 * KH <= m < 3 * KH
                                    else ACTF.Sigmoid)
                            nc.scalar.activation(g[:], pm[ch][:], func,
                                                 bias=b_t[d][m][:],
                                                 scale=1.0 / WSC)
                            gates[ch].append(g)
                    for ch in acts:
                        c0 = ch * C
                        gs = gates[ch]
                        i_g, o_g, u_g, f_g = (gs[0:3], gs[3:6], gs[6:9], gs[9:12])
                        # next-step moving tiles; h(s) is written into them
                        mx_n = state8.tile([128, 2, C], FP8, name=f"mx_{d}_{ch}",
                                           tag=f"mx_{d}_{ch}")
                        hp_n = state8.tile([128, 2, C], FP8, name=f"hp_{d}_{ch}",
                                           tag=f"hp_{d}_{ch}")
                        hsl = h_slices(mx_n, hp_n)
                        cnew = []
                        for k in range(KH):
                            tmp = gatep.tile([128, C], BF16, name="tmp", tag="g")
                            nc.vector.tensor_tensor(tmp[:], i_g[k][:], u_g[k][:],
                                                    ALU.mult)
                            cn = statep.tile([128, C], BF16, name=f"cn_{d}{k}_{ch}",
                                             tag=f"c_{d}{k}_{ch}")
                            if s == 0:
                                nc.vector.tensor_copy(cn[:], tmp[:])
                            else:
                                nc.vector.tensor_tensor(cn[:], f_g[k][:],
                                                        cst[d, ch][k][:], ALU.mult)
                                nc.vector.tensor_tensor(cn[:], cn[:], tmp[:],
                                                        ALU.add)
                            tc_ = gatep.tile([128, C], BF16, name="tc", tag="g")
                            nc.scalar.activation(tc_[:], cn[:], ACTF.Tanh)
                            nc.vector.tensor_tensor(hsl[k], o_g[k][:], tc_[:],
                                                    ALU.mult)
                            cnew.append(cn)
                        cst[d, ch] = cnew
                        mx[d, ch] = mx_n
                        hp[d, ch] = hp_n
                        if d == "u":
                            eq = eqp.tile([1, C], mybir.dt.uint8, name="eq",
                                          tag="eq")
                            nc.vector.tensor_scalar(eq[:], root_t[:, c0:c0 + C],
                                                    float(s), None, ALU.is_equal)
                            mpi = eqp.tile([128, C], mybir.dt.uint8, name="mpi",
                                           tag="mpi", bufs=2)
                            nc.gpsimd.partition_broadcast(mpi[:], eq[:])
                            for k in range(KH):
                                nc.vector.copy_predicated(root_acc[ch][k][:],
                                                          mpi[:], hsl[k])
                        else:
                            if s == 0:
                                for k in range(KH):
                                    end_t[ch][k] = capp.tile(
                                        [128, C], FP8, name=f"end{ch}_{k}",
                                        tag=f"end{ch}_{k}")
                                    nc.vector.tensor_copy(end_t[ch][k][:], hsl[k])
                            if s == L - 1:
                                for k in range(KH):
                                    start_t[ch][k] = capp.tile(
                                        [128, C], FP8, name=f"start{ch}_{k}",
                                        tag=f"start{ch}_{k}")
                                    nc.vector.tensor_copy(start_t[ch][k][:],
                                                          hsl[k])

        # ---- phase 3: pair MLP -----------------------------------------
        with tc.tile_pool(name="mlpw", bufs=1) as mlpw, \
             tc.tile_pool(name="mlpp", bufs=4) as mlpp:
            w1_t = [loadc(mlpw, f"w1{k}", W1[k * 128:(k + 1) * 128, :],
                          [128, DEC_H], BF16) for k in range(K21)]
            w2_t = [loadc(mlpw, f"w2{k}", W2[k * 128:(k + 1) * 128, :],
                          [128, DEC_OUT], BF16) for k in range(M4)]
            for ch in range(NCH):
                c0 = ch * C
                feats = (root_acc[ch] + start_t[ch] + end_t[ch]
                         + [spanT[0][m][:, c0:c0 + C] for m in range(MT)]
                         + [spanT[1][m][:, c0:c0 + C] for m in range(MT)])
                z_t = []
                for m in range(M4):
                    zp = pmm.tile([128, C], FP32, name="zp2", tag="mm")
                    for k in range(K21):
                        fk = feats[k] if isinstance(feats[k], bass.AP) \
                            else feats[k][:]
                        nc.tensor.matmul(zp[:], w1_t[k][:, m * 128:(m + 1) * 128],
                                         fk, start=(k == 0), stop=(k == K21 - 1))
                    z = mlpp.tile([128, C], BF16, name="z", tag="z")
                    nc.scalar.activation(z[:], zp[:], ACTF.Tanh, bias=b1_t[m][:])
                    z_t.append(z)
                opt = pmm.tile([128, C], FP32, name="op", tag="mm")
                op = opt[0:DEC_OUT, :]
                for m in range(M4):
                    nc.tensor.matmul(op, w2_t[m][:], z_t[m][:], start=(m == 0),
                                     stop=(m == M4 - 1))
                osb = mlpp.tile([DEC_OUT, C], FP32, name="osb", tag="osb", bufs=2)
                nc.vector.tensor_scalar(osb[:], op, b2_t[:], None, ALU.add)
                nc.sync.dma_start(out_d[:, c0:c0 + C], osb[:])

    nc.compile()
    _dedupe_ldweights(nc)
    return nc


def _dedupe_ldweights(nc):
    """Remove PE InstLdweights whose weights AP equals the most recently
    retained one with only PE Matmults in between (the PE weight buffer is
    unchanged by other engines). Only wait-free/update-free loads are removed."""
    import concourse.mybir as _mb
    for name, bb in list(nc.bb_map.items()):
        insts = bb.bb.instructions
        out = []
        prev_sig = None
        removed = 0
        for inst in insts:
            tn = type(inst).__name__
            eng = getattr(inst, "engine", None)
            if eng == _mb.EngineType.PE:
                if tn == "InstLdweights":
                    si = inst.sync_info
                    clean = si is None or (not si.on_wait and not si.on_update)
                    try:
                        sig = str(inst.ins[0])
                    except Exception:
                        sig = None
                    if clean and sig is not None and sig == prev_sig:
                        removed += 1
                        continue
                    prev_sig = sig
                elif tn != "InstMatmult":
                    prev_sig = None
            out.append(inst)
        if removed:
            bb.bb.instructions = out


_CACHE = {}


def _get_program(s_up) -> bass.Bass:
    key = tuple(s_up)
    if key not in _CACHE:
        _CACHE[key] = _build_program(s_up=key)
    return _CACHE[key]


def _sort_perm(root_sh):
    """root_sh: [NCORES, NB, PB] int. Returns perm[NCORES, PS] mapping physical
    column j -> local pair index (b*PB + pb), with physical order
    (half, batch, rank-within-half), and s_up (shared across cores)."""
    perm = np.empty((NCORES, PS), np.int64)
    half = PB // 2
    maxr = [0, 0]
    for c in range(NCORES):
        cols = []
        for hf in range(2):
            for b in range(NB):
                order = np.argsort(root_sh[c, b], kind="stable")
                sel = order[hf * half:(hf + 1) * half]
                cols.append(b * PB + sel)
                maxr[hf] = max(maxr[hf], int(root_sh[c, b][sel].max(initial=0)))
        perm[c] = np.concatenate(cols)
    s_up = (maxr[0] + 1, maxr[1] + 1)
    return perm, s_up


def _prep_in_maps(inputs):
    f32 = np.float32
    rooti = np.asarray(inputs["root_idx"]).reshape(NCORES, NB, PB)
    perm, s_up = _sort_perm(rooti)

    node = np.asarray(inputs["node_embs"], f32)
    # [P, L, D] -> per-core column-permuted [L, D, PS]
    node_sh = node.reshape(NCORES, PS, L, D)
    node_sh = np.stack([node_sh[c][perm[c]] for c in range(NCORES)])
    node_T = node_sh.transpose(0, 2, 3, 1)  # [NC, L, D, PS]
    # DoubleRow fp8 layout: pair p block i row q = k-row 256p+128i+q
    na = np.ascontiguousarray(
        node_T[:, :, :2 * 128 * KPA].reshape(NCORES, L, KPA, 2, 128, PS)
        .transpose(0, 1, 2, 4, 3, 5)).astype(fp8)
    nb_ = np.zeros((NCORES, L, 128, PS), fp8)
    nb_[:, :, 0:D - 2 * 128 * KPA] = node_T[:, :, 2 * 128 * KPA:D].astype(fp8)

    tokf = np.asarray(inputs["token_embs"], f32)
    tok_sh = tokf.reshape(NCORES, NB, T, DT).astype(bf16)
    root_perm = np.stack([rooti.reshape(NCORES, PS)[c][perm[c]]
                          for c in range(NCORES)])
    root_sh = root_perm.reshape(NCORES, 1, PS).astype(f32)

    def span_arrays(st, ln):
        st = np.asarray(st).astype(f32)
        ln = np.asarray(ln).astype(f32)
        return st, st + ln + 1.0, 1.0 / (ln + 1.0)

    s1, e1, r1 = span_arrays(inputs["p1_st"], inputs["p1_len"])
    s2, e2, r2 = span_arrays(inputs["p2_st"], inputs["p2_len"])

    def pack_span(a1, a2):
        # [B, PB] x2 -> per-core [2, NB, JT, 128] in permuted order: batch b's
        # jt=0 block holds its low-root half, jt=1 the high half.
        a = np.stack([a1, a2]).reshape(2, NCORES, NB, PB)  # [2, NC, NB, PB]
        outp = np.empty((NCORES, 2, NB, JT, 128), f32)
        half = PB // 2
        for c in range(NCORES):
            pc = perm[c].reshape(2, NB, half)  # [half, b, rank] -> local idx
            for hf in range(2):
                for b in range(NB):
                    idx = pc[hf, b] - b * PB
                    outp[c, :, b, hf] = a[:, c, b, idx]
        return outp

    stp, enp, rcp = pack_span(s1, s2), pack_span(e1, e2), pack_span(r1, r2)
    zp = np.zeros_like(stp)
    sp_all = np.ascontiguousarray(np.stack([stp, enp, rcp, zp], axis=-1))

    def packW5(wiou, wf, uiou, uf):
        w = np.concatenate([np.asarray(wiou, f32),
                            np.asarray(wf, f32)], axis=1) * WSC   # [D, H4]
        u = np.concatenate([np.asarray(uiou, f32),
                            np.asarray(uf, f32)], axis=1) * WSC   # [H, H4]
        wa = np.zeros((KP5, 128, 2, H4), f32)
        for p in range(KPA):                       # x rows 0:768
            wa[p, :, 0] = w[256 * p:256 * p + 128]
            wa[p, :, 1] = w[256 * p + 128:256 * p + 256]
        wa[KPA, 0:64, 0] = w[768:832]              # x tail (rows 64:128 zero)
        wa[KPA, :, 1] = u[256:384]                 # h rows 256:384
        wa[KPA + 1, :, 0] = u[0:128]               # h rows 0:128
        wa[KPA + 1, :, 1] = u[128:256]             # h rows 128:256
        return wa.astype(fp8)

    Wua_h = packW5(inputs["Wiou_u"], inputs["Wf_u"],
                   inputs["Uiou_u"], inputs["Uf_u"])
    Wda_h = packW5(inputs["Wiou_d"], inputs["Wf_d"],
                   inputs["Uiou_d"], inputs["Uf_d"])
    bu_h = np.concatenate([np.asarray(inputs["biou_u"], f32),
                           np.asarray(inputs["bf_u"], f32)]).reshape(M12, 128, 1)
    bd_h = np.concatenate([np.asarray(inputs["biou_d"], f32),
                           np.asarray(inputs["bf_d"], f32)]).reshape(M12, 128, 1)
    W1_h = np.asarray(inputs["W1"], f32).astype(bf16)
    W2_h = np.asarray(inputs["W2"], f32).astype(bf16)
    b1_h = np.asarray(inputs["b1"], f32).reshape(M4, 128, 1)
    b2_h = np.asarray(inputs["b2"], f32).reshape(DEC_OUT, 1)
    ones_h = np.ones((1, 128), bf16)
    iota_h = np.broadcast_to(np.arange(T, dtype=f32), (128, T)).copy()

    in_maps = []
    for c in range(NCORES):
        in_maps.append({
            "node_a": na[c], "node_b": nb_[c], "tok": tok_sh[c],
            "root": root_sh[c], "sp_all": sp_all[c],
            "Wua": Wua_h, "Wda": Wda_h, "W1": W1_h, "W2": W2_h,
            "bu": bu_h, "bd": bd_h, "b1": b1_h, "b2": b2_h,
            "ones": ones_h, "iota_d": iota_h,
        })
    return in_maps, perm, s_up


def _unpermute(outs, perm):
    """outs: list of per-core [PS, 7]; invert the column permutation."""
    full = np.empty((P, DEC_OUT), np.float32)
    for c in range(NCORES):
        full[c * PS + perm[c]] = outs[c]
    return full


def run(inputs, **kwargs):
    in_maps, perm, s_up = _prep_in_maps(inputs)
    nc = _get_program(s_up)
    res = run_bass_kernel_spmd(nc, in_maps, list(range(NCORES)), **kwargs)
    outs = [np.asarray(r["out"], np.float32).T for r in res.results]  # [PS, 7]
    return _unpermute(outs, perm), res


def kernel(**inputs) -> np.ndarray:
    out, _ = run(inputs)
    return out
